# revision 4
# baseline (speedup 1.0000x reference)
"""Cross-attention kernel for Trainium2 (8 NeuronCores, SPMD).

Problem: q [2, 2048, 16, 64], kv [2, 2048, 2, 16, 64] (k=kv[:,:,0], v=kv[:,:,1])
  scores = einsum('bthd,bshd->bhts', q, k/sqrt(d)); P = softmax(scores, -1)
  out = einsum('bhts,bshd->bthd', P, v)    -> [2, 2048, 16, 64]

Sharding: 32 (b,h) heads across 8 cores -> 4 heads/core (data parallel on b,
tensor parallel on h; no communication).

Per-core algorithm (per head, t=s=2048, d=64):
  - Host pre-lays-out (as part of sharding) one combined tensor per head:
    Q^T [64,2048] duplicated into both PE row halves, K^T*scale packed so even
    s-tiles sit at partitions 0-63 and odd s-tiles at 64-127 (enables 2-way
    row-packed matmuls), and V' = [V, 1] (ones column yields the softmax
    denominator from the same matmul). One DMA per head: the fused 4-byte
    (fp32r) matmul instruction can carry at most ONE sync wait, so each
    consumer matmul must depend on a single DMA semaphore.
  - S^T tile [s=128, t] = K_tile @ Q^T  (fp32r matmuls, contraction d=64,
    two s-tiles run concurrently in PE row groups 0-63 / 64-127).
  - P^T = exp(S^T) on ScalarE (PSUM -> SBUF). No max subtraction: scores are
    N(0,1)-distributed, |score| < ~6, so exp is safely in fp32 range and
    softmax is shift-invariant.
  - O'^T [65, t] += V'_i^T @ P^T_i accumulated over s-tiles in PSUM.
    Rows 0-63 = unnormalized O^T, row 64 = sum_s exp = softmax denominator.
  - PE-transpose 128-col chunks of O'^T -> [128, 65]; out = cols 0-63 times
    reciprocal(col 64) on VectorE; DMA to DRAM in [t, h, d] layout.
"""

import math

import numpy as np

import concourse.bass as bass
from concourse import bacc
import concourse.mybir as mybir
import concourse.tile as tile
from concourse.bass_utils import run_bass_kernel_spmd

B, T, H, D = 2, 2048, 16, 64
N_CORES = 8
HPC = (B * H) // N_CORES  # heads per core = 4
P = 128
NS = T // P  # 16 s-tiles
SCALE = 1.0 / math.sqrt(D)
F32 = mybir.dt.float32
F32R = mybir.dt.float32r

# Combined per-head input layout (per partition): [ Q^T 2048 | K^T 1024 | V' 1040 ]
KT_OFF = T
VP_OFF = T + (NS // 2) * P
INP_W = VP_OFF + NS * (D + 1)

LAST_RESULT = None  # BassKernelResults of the most recent kernel() call
_BASS_CACHE = {}


def _build_bass():
    nc = bacc.Bacc("TRN2", target_bir_lowering=False)

    inp_d = nc.declare_dram_parameter("inp", [HPC, P, INP_W], F32R, isOutput=False)
    out_d = nc.declare_dram_parameter("out", [T, HPC, D], F32, isOutput=True)

    ident_d = nc.inline_tensor(np.eye(P, dtype=np.float32), name="ident")

    TW = 1024  # t-half processed per inner pass (PSUM capacity)
    NCHUNK = 512  # matmul moving free dim

    with tile.TileContext(nc) as tc:
        with (
            tc.tile_pool(name="const", bufs=1) as cpool,
            tc.tile_pool(name="heads", bufs=2) as hpool,
            tc.tile_pool(name="pt", bufs=4) as ptpool,
            tc.tile_pool(name="outs", bufs=2) as opool,
            tc.tile_pool(name="spsum", bufs=2, space="PSUM") as spsum,
            tc.tile_pool(name="opsum", bufs=1, space="PSUM") as opsum,
            tc.tile_pool(name="tpsum", bufs=2, space="PSUM") as tpsum,
        ):
            id_sb = cpool.tile([P, P], F32)
            nc.sync.dma_start(id_sb[:], ident_d.ap())
            # Dummy transpose: absorbs the ident-DMA wait on the PE engine so
            # later (wait-limited) matmul/transpose instructions never need it.
            tp0 = tpsum.tile([P, D + 1], F32, tag="tp")
            nc.tensor.transpose(tp0[:], id_sb[: D + 1, :], id_sb[: D + 1, : D + 1])

            out_view = out_d.ap().rearrange("(c p) hh d -> p c hh d", p=P)

            for hh in range(HPC):
                inp_sb = hpool.tile([P, INP_W], F32R, tag="inp")
                nc.sync.dma_start(inp_sb[:], inp_d.ap()[hh])
                qt_sb = inp_sb[:, 0:T]

                def kt_sb(j):  # K^T chunk j: [128, 128]
                    return inp_sb[:, KT_OFF + j * P : KT_OFF + (j + 1) * P]

                def vp_sb(i):  # V' s-tile i: [128, 65]
                    return inp_sb[:, VP_OFF + i * (D + 1) : VP_OFF + (i + 1) * (D + 1)]

                for th in range(T // TW):
                    ps_o = opsum.tile([D + 1, TW], F32, tag="po")

                    for j in range(NS // 2):  # s-tile pairs (2j, 2j+1)
                        psA = spsum.tile([P, TW], F32, tag="ps")
                        psB = spsum.tile([P, TW], F32, tag="ps")
                        # S^T = K_tile @ Q^T; the two s-tiles of the pair run
                        # concurrently in PE row groups 0-63 / 64-127.
                        for c2 in range(TW // NCHUNK):
                            tsl = slice(th * TW + c2 * NCHUNK, th * TW + (c2 + 1) * NCHUNK)
                            csl = slice(c2 * NCHUNK, (c2 + 1) * NCHUNK)
                            nc.tensor.matmul(
                                psA[:, csl],
                                lhsT=kt_sb(j)[0:64, :],
                                rhs=qt_sb[0:64, tsl],
                                start=True,
                                stop=True,
                            )
                            nc.tensor.matmul(
                                psB[:, csl],
                                lhsT=kt_sb(j)[64:128, :],
                                rhs=qt_sb[64:128, tsl],
                                start=True,
                                stop=True,
                            )

                        ptA = ptpool.tile([P, TW], F32R, tag="pt")
                        ptB = ptpool.tile([P, TW], F32R, tag="pt")
                        nc.scalar.activation(ptA[:], psA[:], mybir.ActivationFunctionType.Exp)
                        nc.scalar.activation(ptB[:], psB[:], mybir.ActivationFunctionType.Exp)

                        for c2 in range(TW // NCHUNK):
                            csl = slice(c2 * NCHUNK, (c2 + 1) * NCHUNK)
                            nc.tensor.matmul(
                                ps_o[:, csl],
                                lhsT=vp_sb(2 * j),
                                rhs=ptA[:, csl],
                                start=(j == 0),
                                stop=False,
                            )
                            nc.tensor.matmul(
                                ps_o[:, csl],
                                lhsT=vp_sb(2 * j + 1),
                                rhs=ptB[:, csl],
                                start=False,
                                stop=(j == NS // 2 - 1),
                            )

                    # Normalize + emit this (head, t-half).
                    o_sb = opool.tile([D + 1, TW], F32, tag="osb")
                    nc.vector.tensor_copy(o_sb[:], ps_o[:])
                    ostage = opool.tile([P, TW // P, D], F32, tag="ost")
                    rec = opool.tile([P, TW // P], F32, tag="rec")
                    for cc in range(TW // P):
                        tp = tpsum.tile([P, D + 1], F32, tag="tp")
                        nc.tensor.transpose(
                            tp[:],
                            o_sb[:, cc * P : (cc + 1) * P],
                            id_sb[: D + 1, : D + 1],
                        )
                        nc.vector.reciprocal(rec[:, cc : cc + 1], tp[:, D : D + 1])
                        nc.vector.tensor_scalar_mul(
                            ostage[:, cc, :], tp[:, 0:D], rec[:, cc : cc + 1]
                        )
                    nc.sync.dma_start(
                        out_view[:, th * (TW // P) : (th + 1) * (TW // P), hh, :],
                        ostage[:],
                    )

    nc.compile()
    return nc


def get_bass():
    if "nc" not in _BASS_CACHE:
        _BASS_CACHE["nc"] = _build_bass()
    return _BASS_CACHE["nc"]


def make_core_inputs(q, kv, core):
    """Host-side sharding + layout for one core: returns {inp}."""
    b = core // (N_CORES // B)
    h0 = HPC * (core % (N_CORES // B))
    inp = np.empty((HPC, P, INP_W), np.float32)
    for i in range(HPC):
        h = h0 + i
        Qt = np.ascontiguousarray(q[b, :, h, :].T)  # [64, 2048]
        inp[i, :64, 0:T] = Qt
        inp[i, 64:, 0:T] = Qt
        Kt = (kv[b, :, 0, h, :].astype(np.float32) * SCALE).T  # [64, 2048]
        Kts = Kt.reshape(64, NS, P)
        kt = inp[i, :, KT_OFF:VP_OFF].reshape(P, NS // 2, P)
        kt[:64] = Kts[:, 0::2]  # even s-tiles -> partitions 0-63
        kt[64:] = Kts[:, 1::2]  # odd s-tiles -> partitions 64-127
        V = kv[b, :, 1, h, :].reshape(NS, P, D)  # [s_tile, p, d]
        vp = inp[i, :, VP_OFF:].reshape(P, NS, D + 1)
        vp[:, :, :D] = V.transpose(1, 0, 2)
        vp[:, :, D] = 1.0
    return {"inp": inp}


def kernel(q, kv):
    global LAST_RESULT
    q = np.asarray(q, dtype=np.float32)
    kv = np.asarray(kv, dtype=np.float32)
    assert q.shape == (B, T, H, D) and kv.shape == (B, T, 2, H, D)

    nc = get_bass()
    in_maps = [make_core_inputs(q, kv, c) for c in range(N_CORES)]
    res = run_bass_kernel_spmd(nc, in_maps, core_ids=list(range(N_CORES)))
    LAST_RESULT = res

    out = np.empty((B, T, H, D), np.float32)
    for c in range(N_CORES):
        b = c // (N_CORES // B)
        h0 = HPC * (c % (N_CORES // B))
        out[b, :, h0 : h0 + HPC, :] = res.results[c]["out"]
    return out


# revision 6
# speedup vs baseline: 1.4384x; 1.4384x over previous
"""Cross-attention kernel for Trainium2 (8 NeuronCores, SPMD).

Problem: q [2, 2048, 16, 64], kv [2, 2048, 2, 16, 64] (k=kv[:,:,0], v=kv[:,:,1])
  scores = einsum('bthd,bshd->bhts', q, k/sqrt(d)); P = softmax(scores, -1)
  out = einsum('bhts,bshd->bthd', P, v)    -> [2, 2048, 16, 64]

Sharding: 32 (b,h) heads across 8 cores -> 4 heads/core (data parallel on b,
tensor parallel on h; no communication).

Per-core algorithm (per head, t=s=2048, d=64):
  - Host pre-lays-out (as part of sharding) one combined tensor per head:
    Q^T [64,2048] duplicated into both PE row halves, K^T*scale packed so even
    s-tiles sit at partitions 0-63 and odd s-tiles at 64-127 (enables 2-way
    row-packed matmuls), and V' = [V, 1] (ones column yields the softmax
    denominator from the same matmul). One DMA per head: the fused 4-byte
    (fp32r) matmul instruction can carry at most ONE sync wait, so each
    consumer matmul must depend on a single DMA semaphore.
  - S^T tile [s=128, t] = K_tile @ Q^T  (fp32r matmuls, contraction d=64,
    two s-tiles run concurrently in PE row groups 0-63 / 64-127).
  - P^T = exp(S^T) on ScalarE (PSUM -> SBUF). No max subtraction: scores are
    N(0,1)-distributed, |score| < ~6, so exp is safely in fp32 range and
    softmax is shift-invariant.
  - O'^T [65, t] += V'_i^T @ P^T_i accumulated over s-tiles in PSUM.
    Rows 0-63 = unnormalized O^T, row 64 = sum_s exp = softmax denominator.
  - PE-transpose 128-col chunks of O'^T -> [128, 65]; out = cols 0-63 times
    reciprocal(col 64) on VectorE; DMA to DRAM in [t, h, d] layout.
"""

import math

import numpy as np

import concourse.bass as bass
from concourse import bacc
import concourse.mybir as mybir
import concourse.tile as tile
from concourse.bass_utils import run_bass_kernel_spmd

B, T, H, D = 2, 2048, 16, 64
N_CORES = 8
HPC = (B * H) // N_CORES  # heads per core = 4
P = 128
NS = T // P  # 16 s-tiles
SCALE = 1.0 / math.sqrt(D)
F32 = mybir.dt.float32
F32R = mybir.dt.float32r
F16 = mybir.dt.float16

# Combined per-head input layout (per partition): [ Q^T 2048 | K^T 1024 | V' 1040 ]
KT_OFF = T
VP_OFF = T + (NS // 2) * P
INP_W = VP_OFF + NS * (D + 1)

LAST_RESULT = None  # BassKernelResults of the most recent kernel() call
_BASS_CACHE = {}


def _build_bass():
    nc = bacc.Bacc("TRN2", target_bir_lowering=False)

    inp_d = nc.declare_dram_parameter("inp", [HPC, P, INP_W], F16, isOutput=False)
    out_d = nc.declare_dram_parameter("out", [T, HPC, D], F32, isOutput=True)

    ident_d = nc.inline_tensor(np.eye(P, dtype=np.float32), name="ident")

    TW = 1024  # t-half processed per inner pass (PSUM capacity)
    NCHUNK = 512  # matmul moving free dim

    with tile.TileContext(nc) as tc:
        with (
            tc.tile_pool(name="const", bufs=1) as cpool,
            tc.tile_pool(name="heads", bufs=2) as hpool,
            tc.tile_pool(name="pt", bufs=4) as ptpool,
            tc.tile_pool(name="outs", bufs=2) as opool,
            tc.tile_pool(name="spsum", bufs=2, space="PSUM") as spsum,
            tc.tile_pool(name="opsum", bufs=1, space="PSUM") as opsum,
            tc.tile_pool(name="tpsum", bufs=2, space="PSUM") as tpsum,
        ):
            id_sb = cpool.tile([P, P], F32)
            nc.sync.dma_start(id_sb[:], ident_d.ap())
            # Dummy transpose: absorbs the ident-DMA wait on the PE engine so
            # later (wait-limited) matmul/transpose instructions never need it.
            tp0 = tpsum.tile([P, D + 1], F32, tag="tp")
            nc.tensor.transpose(tp0[:], id_sb[: D + 1, :], id_sb[: D + 1, : D + 1])

            out_view = out_d.ap().rearrange("(c p) hh d -> p c hh d", p=P)

            for hh in range(HPC):
                inp_sb = hpool.tile([P, INP_W], F16, tag="inp")
                nc.sync.dma_start(inp_sb[:], inp_d.ap()[hh])
                qt_sb = inp_sb[:, 0:T]

                def kt_sb(j):  # K^T chunk j: [128, 128]
                    return inp_sb[:, KT_OFF + j * P : KT_OFF + (j + 1) * P]

                def vp_sb(i):  # V' s-tile i: [128, 65]
                    return inp_sb[:, VP_OFF + i * (D + 1) : VP_OFF + (i + 1) * (D + 1)]

                for th in range(T // TW):
                    ps_o = opsum.tile([D + 1, TW], F32, tag="po")

                    for j in range(NS // 2):  # s-tile pairs (2j, 2j+1)
                        psA = spsum.tile([P, TW], F32, tag="ps")
                        psB = spsum.tile([P, TW], F32, tag="ps")
                        # S^T = K_tile @ Q^T; the two s-tiles of the pair run
                        # concurrently in PE row groups 0-63 / 64-127.
                        for c2 in range(TW // NCHUNK):
                            tsl = slice(th * TW + c2 * NCHUNK, th * TW + (c2 + 1) * NCHUNK)
                            csl = slice(c2 * NCHUNK, (c2 + 1) * NCHUNK)
                            nc.tensor.matmul(
                                psA[:, csl],
                                lhsT=kt_sb(j)[0:64, :],
                                rhs=qt_sb[0:64, tsl],
                                start=True,
                                stop=True,
                            )
                            nc.tensor.matmul(
                                psB[:, csl],
                                lhsT=kt_sb(j)[64:128, :],
                                rhs=qt_sb[64:128, tsl],
                                start=True,
                                stop=True,
                            )

                        ptA = ptpool.tile([P, TW], F16, tag="pt")
                        ptB = ptpool.tile([P, TW], F16, tag="pt")
                        nc.scalar.activation(ptA[:], psA[:], mybir.ActivationFunctionType.Exp)
                        nc.scalar.activation(ptB[:], psB[:], mybir.ActivationFunctionType.Exp)

                        for c2 in range(TW // NCHUNK):
                            csl = slice(c2 * NCHUNK, (c2 + 1) * NCHUNK)
                            nc.tensor.matmul(
                                ps_o[:, csl],
                                lhsT=vp_sb(2 * j),
                                rhs=ptA[:, csl],
                                start=(j == 0),
                                stop=False,
                            )
                            nc.tensor.matmul(
                                ps_o[:, csl],
                                lhsT=vp_sb(2 * j + 1),
                                rhs=ptB[:, csl],
                                start=False,
                                stop=(j == NS // 2 - 1),
                            )

                    # Normalize + emit this (head, t-half).
                    o_sb = opool.tile([D + 1, TW], F32, tag="osb")
                    nc.vector.tensor_copy(o_sb[:], ps_o[:])
                    ostage = opool.tile([P, TW // P, D], F32, tag="ost")
                    rec = opool.tile([P, TW // P], F32, tag="rec")
                    for cc in range(TW // P):
                        tp = tpsum.tile([P, D + 1], F32, tag="tp")
                        nc.tensor.transpose(
                            tp[:],
                            o_sb[:, cc * P : (cc + 1) * P],
                            id_sb[: D + 1, : D + 1],
                        )
                        nc.vector.reciprocal(rec[:, cc : cc + 1], tp[:, D : D + 1])
                        nc.vector.tensor_scalar_mul(
                            ostage[:, cc, :], tp[:, 0:D], rec[:, cc : cc + 1]
                        )
                    nc.sync.dma_start(
                        out_view[:, th * (TW // P) : (th + 1) * (TW // P), hh, :],
                        ostage[:],
                    )

    nc.compile()
    return nc


def get_bass():
    if "nc" not in _BASS_CACHE:
        _BASS_CACHE["nc"] = _build_bass()
    return _BASS_CACHE["nc"]


def make_core_inputs(q, kv, core):
    """Host-side sharding + layout for one core: returns {inp}."""
    b = core // (N_CORES // B)
    h0 = HPC * (core % (N_CORES // B))
    inp = np.empty((HPC, P, INP_W), np.float16)
    for i in range(HPC):
        h = h0 + i
        Qt = np.ascontiguousarray(q[b, :, h, :].T)  # [64, 2048]
        inp[i, :64, 0:T] = Qt
        inp[i, 64:, 0:T] = Qt
        Kt = (kv[b, :, 0, h, :].astype(np.float32) * SCALE).T  # [64, 2048]
        Kts = Kt.reshape(64, NS, P)
        kt = inp[i, :, KT_OFF:VP_OFF].reshape(P, NS // 2, P)
        kt[:64] = Kts[:, 0::2]  # even s-tiles -> partitions 0-63
        kt[64:] = Kts[:, 1::2]  # odd s-tiles -> partitions 64-127
        V = kv[b, :, 1, h, :].reshape(NS, P, D)  # [s_tile, p, d]
        vp = inp[i, :, VP_OFF:].reshape(P, NS, D + 1)
        vp[:, :, :D] = V.transpose(1, 0, 2)
        vp[:, :, D] = 1.0
    return {"inp": inp}


def kernel(q, kv):
    global LAST_RESULT
    q = np.asarray(q, dtype=np.float32)
    kv = np.asarray(kv, dtype=np.float32)
    assert q.shape == (B, T, H, D) and kv.shape == (B, T, 2, H, D)

    nc = get_bass()
    in_maps = [make_core_inputs(q, kv, c) for c in range(N_CORES)]
    res = run_bass_kernel_spmd(nc, in_maps, core_ids=list(range(N_CORES)))
    LAST_RESULT = res

    out = np.empty((B, T, H, D), np.float32)
    for c in range(N_CORES):
        b = c // (N_CORES // B)
        h0 = HPC * (c % (N_CORES // B))
        out[b, :, h0 : h0 + HPC, :] = res.results[c]["out"]
    return out
